# revision 8
# baseline (speedup 1.0000x reference)
"""MultiHeadAttentionLayer (head-mixing per-position attention) on 8 NeuronCores.

Bass/Tile kernel. Sharding: data-parallel over the flattened batch*seq
position axis (N*L = 16384 positions -> 2048 per core). The reference
"attention" mixes HEADS within each position (einsum nlhd,nled->nlhe), so
positions are fully independent: no collectives.

Per core pipeline (bf16 compute, fp32 PSUM accumulation):
  - PE transposes each input position-tile so the contraction dim (feature)
    lands on partitions, then runs the q/k/v projections with the weight
    matrices as the moving operand. Biases enter via a ones-row matmul.
  - DVE computes the per-position 16x16 head-mix logits with broadcast-AP
    multiplies and binary-tree segmented reductions (bf16 2x mode), the
    softmax normalization, and the attn*v contraction the same way.
  - ACT does PSUM->SBUF copies (with downcast) and the softmax exp.
  - PE transposes ctx and applies the output projection; the v-bias is
    folded into the output bias host-side (softmax rows sum to 1).
"""

import sys

sys.path.insert(0, "/opt/trn_rl_repo")

import numpy as np

N, L, HID, EMB, NH, HD = 4, 4096, 1024, 1024, 16, 64
NCORES = 8
P = (N * L) // NCORES  # positions per core = 2048
NPT = P // 128  # 16 position-tiles per core
BLK = 8  # position-tiles per block
NBLK = NPT // BLK

_CACHE = {}


def _bcast(ap_slice, dim, n):
    """Insert a stride-0 (broadcast) axis of length n at free-dim position
    `dim` (0 = partition dim) of an existing AP."""
    import concourse.bass as bass

    a = [list(x) for x in ap_slice.ap]
    newap = a[:dim] + [[0, n]] + a[dim:]
    return bass.AP(tensor=ap_slice.tensor, offset=ap_slice.offset, ap=newap)


def _build_bass():
    import concourse.bass as bass
    import concourse.tile as tile
    from concourse import bacc, mybir
    from concourse.masks import make_identity

    f32 = mybir.dt.float32
    bf = mybir.dt.bfloat16
    AF = mybir.ActivationFunctionType

    nc = bacc.Bacc()

    xq = nc.dram_tensor("xq", [P, HID], f32, kind="ExternalInput")
    xk = nc.dram_tensor("xk", [P, HID], f32, kind="ExternalInput")
    xv = nc.dram_tensor("xv", [P, HID], f32, kind="ExternalInput")
    # wq/wk rows 0..HID-1 = W.T (so [h, e]); last row = bias.
    wq = nc.dram_tensor("wq", [HID + 1, EMB], bf, kind="ExternalInput")
    wk = nc.dram_tensor("wk", [HID + 1, EMB], bf, kind="ExternalInput")
    wv = nc.dram_tensor("wv", [HID + 1, EMB], bf, kind="ExternalInput")
    # wo rows 0..EMB-1 = Wo.T ([hd, o]); last row = bo.
    wo = nc.dram_tensor("wo", [EMB + 1, HID], bf, kind="ExternalInput")
    out = nc.dram_tensor("out", [P, HID], f32, kind="ExternalOutput")

    with tile.TileContext(nc) as tc:
        with (
            tc.tile_pool(name="wpool", bufs=1) as wpool,
            tc.tile_pool(name="constp", bufs=1) as constp,
            tc.tile_pool(name="stage", bufs=2) as stagep,
            tc.tile_pool(name="xtp", bufs=2) as xtp,
            tc.tile_pool(name="qkv", bufs=1) as qkvp,
            tc.tile_pool(name="attnp", bufs=1) as attnp,
            tc.tile_pool(name="tmpp", bufs=1) as tmpp,
            tc.tile_pool(name="treep", bufs=1) as treep,
            tc.tile_pool(name="outp", bufs=2) as outp,
            tc.tile_pool(name="ps_t", bufs=2, space=bass.MemorySpace.PSUM) as ps_t,
            tc.tile_pool(name="ps_p", bufs=2, space=bass.MemorySpace.PSUM) as ps_p,
        ):
            # ---- weights resident in SBUF (bf16) ----
            def load_w(dram, rows, tag):
                t = wpool.tile([128, rows // 128, dram.shape[1]], bf, tag=tag)
                nc.sync.dma_start(
                    t[:],
                    dram[0:rows, :].rearrange("(kt p) e -> p kt e", p=128),
                )
                return t

            w_q = load_w(wq, HID, "wq")
            w_k = load_w(wk, HID, "wk")
            w_v = load_w(wv, HID, "wv")
            w_o = load_w(wo, EMB, "wo")
            b_q = wpool.tile([1, EMB], bf)
            nc.sync.dma_start(b_q[:], wq[HID : HID + 1, :])
            b_k = wpool.tile([1, EMB], bf)
            nc.sync.dma_start(b_k[:], wk[HID : HID + 1, :])
            b_v = wpool.tile([1, EMB], bf)
            nc.sync.dma_start(b_v[:], wv[HID : HID + 1, :])
            b_o = wpool.tile([1, HID], bf)
            nc.sync.dma_start(b_o[:], wo[EMB : EMB + 1, :])

            ident = constp.tile([128, 128], f32)
            make_identity(nc, ident[:])
            ident_bf = constp.tile([128, 128], bf)
            make_identity(nc, ident_bf[:])
            ones1 = constp.tile([1, 128], bf)
            nc.vector.memset(ones1[:], 1.0)

            for blk in range(NBLK):
                q_sb = qkvp.tile([128, BLK, NH, HD], bf, tag="q")
                k_sb = qkvp.tile([128, BLK, NH, HD], bf, tag="k")
                v_sb = qkvp.tile([128, BLK, NH, HD], bf, tag="v")

                # ---- projections, one position-tile at a time ----
                for j in range(BLK):
                    pt = blk * BLK + j
                    for xdram, wt, brow, dst in (
                        (xq, w_q, b_q, q_sb),
                        (xk, w_k, b_k, k_sb),
                        (xv, w_v, b_v, v_sb),
                    ):
                        st = stagep.tile([128, HID], f32, tag="stage")
                        nc.sync.dma_start(st[:], xdram[pt * 128 : (pt + 1) * 128, :])
                        pst = ps_t.tile([128, 8, 128], f32, tag="pst")
                        for t8 in range(8):
                            nc.tensor.transpose(
                                pst[:, t8, :],
                                st[:, t8 * 128 : (t8 + 1) * 128],
                                ident[:],
                            )
                        xt = xtp.tile([128, 8, 128], bf, tag="xt")
                        nc.scalar.copy(xt[:], pst[:])
                        psp = ps_p.tile([128, EMB], f32, tag="psp")
                        for half in range(2):
                            sl = slice(half * 512, (half + 1) * 512)
                            for kt in range(8):
                                nc.tensor.matmul(
                                    psp[:, sl],
                                    xt[:, kt, :],
                                    wt[:, kt, sl],
                                    start=(kt == 0),
                                    stop=(brow is None and kt == 7),
                                )
                            if brow is not None:
                                nc.tensor.matmul(
                                    psp[:, sl],
                                    ones1[:],
                                    brow[:, sl],
                                    start=False,
                                    stop=True,
                                )
                        nc.scalar.copy(
                            dst[:].rearrange("p b h d -> p b (h d)")[:, j, :], psp[:]
                        )

                # ---- head-mix logits: logits[p,b,h,e] = sum_d q[h,d]*k[e,d] ----
                logits = attnp.tile([128, BLK, NH, NH], bf, tag="logits")
                for h in range(NH):
                    tmp = tmpp.tile([128, BLK, NH, HD], bf, tag="tmp")
                    q_h = q_sb[:, :, h, :]  # [128, BLK, HD]
                    nc.vector.tensor_mul(tmp[:], k_sb[:], _bcast(q_h, 2, NH))
                    cur = tmp[:]
                    s = HD
                    while s > 2:
                        nxt = treep.tile(
                            [128, BLK, NH, s // 2], bf, tag=f"tree{NH * s // 2}"
                        )
                        nc.vector.tensor_add(
                            nxt[:], cur[:, :, :, 0 : s // 2], cur[:, :, :, s // 2 : s]
                        )
                        cur = nxt[:]
                        s //= 2
                    nc.vector.tensor_add(
                        logits[:, :, h, :], cur[:, :, :, 0], cur[:, :, :, 1]
                    )

                # ---- softmax over e (scale 1/sqrt(HD); range is small, no max-sub) ----
                expv = attnp.tile([128, BLK, NH, NH], bf, tag="expv")
                nc.scalar.activation(
                    expv[:], logits[:], AF.Exp, bias=0.0, scale=1.0 / np.sqrt(HD)
                )
                den = attnp.tile([128, BLK, NH], f32, tag="den")
                nc.vector.tensor_reduce(
                    den[:], expv[:], axis=mybir.AxisListType.X, op=mybir.AluOpType.add
                )
                rec = attnp.tile([128, BLK, NH], f32, tag="rec")
                nc.vector.reciprocal(rec[:], den[:])
                recb = attnp.tile([128, BLK, NH], bf, tag="recb")
                nc.vector.tensor_copy(recb[:], rec[:])
                attn = attnp.tile([128, BLK, NH, NH], bf, tag="attn")
                nc.vector.tensor_mul(attn[:], expv[:], _bcast(recb[:], 3, NH))

                # ---- ctx[p,b,h,d] = sum_e attn[h,e] * v[e,d] ----
                ctx = qkvp.tile([128, BLK, NH, HD], bf, tag="ctx")
                for h in range(NH):
                    tmp2 = tmpp.tile([128, BLK, NH, HD], bf, tag="tmp")
                    a_h = attn[:, :, h, :]  # [128, BLK, NH(e)]
                    nc.vector.tensor_mul(tmp2[:], v_sb[:], _bcast(a_h, 3, HD))
                    cur = tmp2[:]
                    s = NH
                    while s > 2:
                        nxt = treep.tile(
                            [128, BLK, s // 2, HD], bf, tag=f"tree{HD * s // 2}"
                        )
                        nc.vector.tensor_add(
                            nxt[:], cur[:, :, 0 : s // 2, :], cur[:, :, s // 2 : s, :]
                        )
                        cur = nxt[:]
                        s //= 2
                    nc.vector.tensor_add(
                        ctx[:, :, h, :], cur[:, :, 0, :], cur[:, :, 1, :]
                    )

                # ---- output projection ----
                cflat = ctx[:].rearrange("p b h d -> p b (h d)")
                for j in range(BLK):
                    pt = blk * BLK + j
                    pst2 = ps_t.tile([128, 8, 128], bf, tag="pst")
                    for t8 in range(8):
                        nc.tensor.transpose(
                            pst2[:, t8, :],
                            cflat[:, j, t8 * 128 : (t8 + 1) * 128],
                            ident_bf[:],
                        )
                    ctxt = xtp.tile([128, 8, 128], bf, tag="xt")
                    nc.scalar.copy(ctxt[:], pst2[:])
                    pso = ps_p.tile([128, HID], f32, tag="psp")
                    for half in range(2):
                        sl = slice(half * 512, (half + 1) * 512)
                        for kt in range(8):
                            nc.tensor.matmul(
                                pso[:, sl],
                                ctxt[:, kt, :],
                                w_o[:, kt, sl],
                                start=(kt == 0),
                                stop=False,
                            )
                        nc.tensor.matmul(
                            pso[:, sl], ones1[:], b_o[:, sl], start=False, stop=True
                        )
                    ob = outp.tile([128, HID], f32, tag="ob")
                    nc.scalar.copy(ob[:], pso[:])
                    nc.sync.dma_start(out[pt * 128 : (pt + 1) * 128, :], ob[:])

    nc.compile()
    return nc


def get_nc():
    if "nc" not in _CACHE:
        _CACHE["nc"] = _build_bass()
    return _CACHE["nc"]


def make_in_maps(Q, K, V, Wq, bq, Wk, bk, Wv, bv, Wo, bo):
    import ml_dtypes

    bfnp = ml_dtypes.bfloat16
    f32 = np.float32
    Xq = np.ascontiguousarray(np.asarray(Q, f32).reshape(-1, HID))
    Xk = np.ascontiguousarray(np.asarray(K, f32).reshape(-1, HID))
    Xv = np.ascontiguousarray(np.asarray(V, f32).reshape(-1, HID))
    Wq, Wk, Wv, Wo = (np.asarray(a, f32) for a in (Wq, Wk, Wv, Wo))
    bq, bk, bv, bo = (np.asarray(a, f32) for a in (bq, bk, bv, bo))
    wq_np = np.ascontiguousarray(np.vstack([Wq.T, bq[None]]).astype(bfnp))
    wk_np = np.ascontiguousarray(np.vstack([Wk.T, bk[None]]).astype(bfnp))
    wv_np = np.ascontiguousarray(np.vstack([Wv.T, bv[None]]).astype(bfnp))
    wo_np = np.ascontiguousarray(np.vstack([Wo.T, bo[None]]).astype(bfnp))
    return [
        {
            "xq": Xq[i * P : (i + 1) * P],
            "xk": Xk[i * P : (i + 1) * P],
            "xv": Xv[i * P : (i + 1) * P],
            "wq": wq_np,
            "wk": wk_np,
            "wv": wv_np,
            "wo": wo_np,
        }
        for i in range(NCORES)
    ]


def kernel(Q, K, V, Wq, bq, Wk, bk, Wv, bv, Wo, bo):
    from concourse.bass_utils import run_bass_kernel_spmd

    in_maps = make_in_maps(Q, K, V, Wq, bq, Wk, bk, Wv, bv, Wo, bo)
    nc = get_nc()
    res = run_bass_kernel_spmd(nc, in_maps, core_ids=list(range(NCORES)))
    outs = [res.results[i]["out"] for i in range(NCORES)]
    return np.concatenate(outs, axis=0).reshape(N, L, HID).astype(np.float32)
